# Initial kernel scaffold
#
"""Bass/TRN2 kernel for nn_MaxAttention (ragged masked max-pool + attention).

Reference semantics: x = code_output.reshape(B, H, S) (raw reshape, not a
transpose); masked max/argmax over the last axis with column mask j < len[b];
attn = pool @ fc_w.T; plus an attribution scatter for samples 0..9.

Strategy: data-parallel over batch on 8 cores, length-sorted slot assignment
(8 same-slot samples across cores share one compile-time width = max group
length), host packs per-core inputs in SBUF layout with -FLT_MAX padding so
no on-device masking is needed. The device does the memory-bound work: the
masked row-max over every valid element, plus top-1 argmax for the two slots
holding samples 0..9. Tiny epilogue (11-wide matmul, normalize, scatter of
10x512 values) runs on host.
"""

import numpy as np

import concourse.bacc as bacc
import concourse.tile as tile
from concourse import mybir
from concourse.bass_utils import run_bass_kernel_spmd

B, S, H, LABS = 128, 2048, 512, 11
NCORES = 8
NSLOTS = B // NCORES  # 16 samples (slots) per core
RB = 4                # 512 rows = 4 rowblocks of 128 partitions
GRAN = 32             # slot width granule (elements)
PAD = np.float32(np.finfo(np.float32).min)

_program_cache: dict = {}


def _assign_slots(lengths: np.ndarray, need_argmax: bool):
    """Return (slot_samples [16][8], widths [16], argmax_slots list).

    Samples 0..9 (which need argmax for the attribution scatter) are pinned
    to slots 0..1; everything else is length-sorted descending so the 8
    samples sharing a slot (one per core) have similar lengths, minimizing
    the padding to the per-slot compile-time width.
    """
    lengths = lengths.astype(np.int64)
    if need_argmax:
        spec = sorted(range(10), key=lambda b: -lengths[b])
        slot0 = spec[:8]
        left = spec[8:]  # 2 specials for slot 1
        others = sorted(range(10, B), key=lambda b: -lengths[b])
        m = max(lengths[b] for b in left)
        below = [b for b in others if lengths[b] <= m]
        sel = below[:6]
        if len(sel) < 6:
            above = [b for b in others if lengths[b] > m]
            sel += above[len(above) - (6 - len(sel)):]
        slot1 = left + sel
        used = set(slot1)
        rest = [b for b in others if b not in used]  # 112, sorted desc
        slots = [slot0, slot1] + [rest[8 * i: 8 * i + 8] for i in range(14)]
        argmax_slots = [0, 1]
    else:
        order = sorted(range(B), key=lambda b: -lengths[b])
        slots = [order[8 * i: 8 * i + 8] for i in range(NSLOTS)]
        argmax_slots = []
    widths = []
    for sl in slots:
        w = int(max(lengths[b] for b in sl))
        w = max(GRAN, -(-w // GRAN) * GRAN)
        widths.append(w)
    return slots, widths, argmax_slots


def _build_program(widths, argmax_slots):
    """One SPMD program shared by all 8 cores: per slot, DMA [128, 4*W] and
    row-max (plus top-1 argmax for argmax slots)."""
    widths = list(widths)
    ctotal = 4 * sum(widths)
    n_arg = len(argmax_slots)

    nc = bacc.Bacc("TRN2", debug=False, num_devices=NCORES)
    xin = nc.dram_tensor("xin", [128, ctotal], mybir.dt.float32,
                         kind="ExternalInput").ap()
    pool_out = nc.dram_tensor("pool_out", [128, RB * NSLOTS], mybir.dt.float32,
                              kind="ExternalOutput").ap()
    idx_out = None
    if n_arg:
        idx_out = nc.dram_tensor("idx_out", [128, RB * n_arg], mybir.dt.uint32,
                                 kind="ExternalOutput").ap()

    with tile.TileContext(nc) as tc:
        with (
            tc.tile_pool(name="xp", bufs=3) as xp,
            tc.tile_pool(name="acc", bufs=1) as acc,
            tc.tile_pool(name="small", bufs=4) as small,
        ):
            pool_t = acc.tile([128, RB * NSLOTS], mybir.dt.float32)
            idx_t = acc.tile([128, RB * n_arg], mybir.dt.uint32) if n_arg else None
            off = 0
            for k, w in enumerate(widths):
                xt = xp.tile([128, RB * w], mybir.dt.float32, tag="x")
                nc.sync.dma_start(out=xt, in_=xin[:, off:off + RB * w])
                if k in argmax_slots:
                    j = argmax_slots.index(k)
                    for rb in range(RB):
                        sl = xt[:, rb * w:(rb + 1) * w]
                        m8 = small.tile([128, 8], mybir.dt.float32, tag="m8")
                        i8 = small.tile([128, 8], mybir.dt.uint32, tag="i8")
                        nc.vector.max(out=m8, in_=sl)
                        nc.vector.max_index(out=i8, in_max=m8, in_values=sl)
                        col = k * RB + rb
                        nc.vector.tensor_copy(out=pool_t[:, col:col + 1],
                                              in_=m8[:, 0:1])
                        jcol = j * RB + rb
                        nc.vector.tensor_copy(out=idx_t[:, jcol:jcol + 1],
                                              in_=i8[:, 0:1])
                else:
                    nc.vector.tensor_reduce(
                        out=pool_t[:, k * RB:(k + 1) * RB],
                        in_=xt.rearrange("p (r w) -> p r w", r=RB),
                        op=mybir.AluOpType.max,
                        axis=mybir.AxisListType.X,
                    )
                off += RB * w
            nc.sync.dma_start(out=pool_out, in_=pool_t)
            if n_arg:
                nc.sync.dma_start(out=idx_out, in_=idx_t)
    nc.compile()
    return nc


def _get_program(widths, argmax_slots):
    key = (tuple(widths), tuple(argmax_slots))
    if key not in _program_cache:
        _program_cache[key] = _build_program(widths, argmax_slots)
    return _program_cache[key]


def _pack_inputs(xv, lengths, slots, widths):
    """Build per-core [128, C] buffers in SBUF layout (partition p holds
    x-rows {p, 128+p, 256+p, 384+p}), padded with -FLT_MAX past each length."""
    ctotal = 4 * sum(widths)
    bufs = [np.empty((128, ctotal), np.float32) for _ in range(NCORES)]
    for k, (sl, w) in enumerate(zip(slots, widths)):
        off = 4 * sum(widths[:k])
        for c, b in enumerate(sl):
            ln = int(lengths[b])
            block = bufs[c][:, off:off + 4 * w].reshape(128, RB, w)
            # xv[b, rb, p, :] -> block[p, rb, :]
            block[:, :, :ln] = xv[b, :, :, :ln].transpose(1, 0, 2)
            if ln < w:
                block[:, :, ln:] = PAD
    return bufs


def kernel(find_contri, code_output, lengths, contri, fc_w):
    fc = int(np.asarray(find_contri))
    co = np.ascontiguousarray(np.asarray(code_output, dtype=np.float32))
    lengths = np.asarray(lengths).astype(np.int64)
    fc_w = np.asarray(fc_w, dtype=np.float32)

    need_argmax = bool(fc)
    slots, widths, argmax_slots = _assign_slots(lengths, need_argmax)
    nc = _get_program(widths, argmax_slots)

    xv = co.reshape(B, RB, 128, S)  # x[b, rb*128+p, j] = xv[b, rb, p, j]
    bufs = _pack_inputs(xv, lengths, slots, widths)
    in_maps = [{"xin": bufs[c]} for c in range(NCORES)]

    res = run_bass_kernel_spmd(nc, in_maps, core_ids=list(range(NCORES)))

    pool = np.empty((B, H), np.float32)
    idx = np.zeros((B, H), np.int64)
    for k, sl in enumerate(slots):
        for c, b in enumerate(sl):
            pr = res.results[c]["pool_out"]  # [128, 64]
            pool[b] = pr[:, k * RB:(k + 1) * RB].T.reshape(H)
            if k in argmax_slots:
                j = argmax_slots.index(k)
                ir = res.results[c]["idx_out"]  # [128, 4*n_arg]
                idx[b] = ir[:, j * RB:(j + 1) * RB].T.reshape(H).astype(np.int64)

    attn = pool @ fc_w.T  # [B, 11] f32
    if not fc:
        return attn

    contri = np.asarray(contri, dtype=np.float32)
    p10 = pool[:10]
    con = np.maximum(p10[:, None, :] * fc_w[None, :, :], 0.0) + np.float32(1e-4)
    con = con / con.sum(axis=2, keepdims=True, dtype=np.float32) * np.float32(100.0)
    total = con.sum(axis=1, dtype=np.float32)  # [10, H]
    ch2 = np.zeros((B, S), np.float32)
    for i in range(10):
        np.add.at(ch2[i], idx[i], total[i])
    contri_out = np.concatenate([contri, ch2[..., None]], axis=2)
    return attn, contri_out


# revision 3
# speedup vs baseline: 1.6248x; 1.6248x over previous
"""Bass/TRN2 kernel for nn_MaxAttention (ragged masked max-pool + attention).

Reference semantics: x = code_output.reshape(B, H, S) (raw reshape, not a
transpose); masked max/argmax over the last axis with column mask j < len[b];
attn = pool @ fc_w.T; plus an attribution scatter for samples 0..9.

Strategy: data-parallel over batch on 8 cores, length-sorted slot assignment
(8 same-slot samples across cores share one compile-time width = max group
length), host packs per-core inputs in SBUF layout with -FLT_MAX padding so
no on-device masking is needed. The device does the memory-bound work: the
masked row-max over every valid element, plus top-1 argmax for the two slots
holding samples 0..9. Tiny epilogue (11-wide matmul, normalize, scatter of
10x512 values) runs on host.
"""

import numpy as np

import concourse.bacc as bacc
import concourse.tile as tile
from concourse import mybir
from concourse.bass_utils import run_bass_kernel_spmd

B, S, H, LABS = 128, 2048, 512, 11
NCORES = 8
NSLOTS = B // NCORES  # 16 samples (slots) per core
RB = 4                # 512 rows = 4 rowblocks of 128 partitions
GRAN = 32             # slot width granule (elements)
PAD = np.float32(np.finfo(np.float32).min)

_program_cache: dict = {}


def _assign_slots(lengths: np.ndarray, need_argmax: bool):
    """Return (slot_samples [16][8], widths [16], argmax_slots list).

    Samples 0..9 (which need argmax for the attribution scatter) are pinned
    to slots 0..1; everything else is length-sorted descending so the 8
    samples sharing a slot (one per core) have similar lengths, minimizing
    the padding to the per-slot compile-time width.
    """
    lengths = lengths.astype(np.int64)
    if need_argmax:
        spec = sorted(range(10), key=lambda b: -lengths[b])
        slot0 = spec[:8]
        left = spec[8:]  # 2 specials for slot 1
        others = sorted(range(10, B), key=lambda b: -lengths[b])
        m = max(lengths[b] for b in left)
        below = [b for b in others if lengths[b] <= m]
        sel = below[:6]
        if len(sel) < 6:
            above = [b for b in others if lengths[b] > m]
            sel += above[len(above) - (6 - len(sel)):]
        slot1 = left + sel
        used = set(slot1)
        rest = [b for b in others if b not in used]  # 112, sorted desc
        slots = [slot0, slot1] + [rest[8 * i: 8 * i + 8] for i in range(14)]
        argmax_slots = [0, 1]
    else:
        order = sorted(range(B), key=lambda b: -lengths[b])
        slots = [order[8 * i: 8 * i + 8] for i in range(NSLOTS)]
        argmax_slots = []
    widths = []
    for sl in slots:
        w = int(max(lengths[b] for b in sl))
        w = max(GRAN, -(-w // GRAN) * GRAN)
        widths.append(w)
    return slots, widths, argmax_slots


def _build_program(widths, argmax_slots, n_repeat=1):
    """One SPMD program shared by all 8 cores: per slot, DMA [128, 4*W] and
    row-max (plus top-1 argmax for argmax slots). n_repeat>1 wraps the body
    in a hardware loop (benchmarking only)."""
    from contextlib import nullcontext
    widths = list(widths)
    ctotal = 4 * sum(widths)
    n_arg = len(argmax_slots)

    nc = bacc.Bacc("TRN2", debug=False, num_devices=NCORES)
    xin = nc.dram_tensor("xin", [128, ctotal], mybir.dt.float32,
                         kind="ExternalInput").ap()
    pool_out = nc.dram_tensor("pool_out", [128, RB * NSLOTS], mybir.dt.float32,
                              kind="ExternalOutput").ap()
    idx_out = None
    if n_arg:
        idx_out = nc.dram_tensor("idx_out", [128, RB * n_arg], mybir.dt.uint32,
                                 kind="ExternalOutput").ap()

    with tile.TileContext(nc) as tc:
        with (
            tc.tile_pool(name="xp", bufs=3) as xp,
            tc.tile_pool(name="acc", bufs=1) as acc,
            tc.tile_pool(name="small", bufs=4) as small,
            tc.For_i(0, n_repeat, 1) if n_repeat > 1 else nullcontext(),
        ):
            pool_t = acc.tile([128, RB * NSLOTS], mybir.dt.float32)
            idx_t = (acc.tile([128, RB * n_arg], mybir.dt.uint32, name="idx_t")
                     if n_arg else None)
            off = 0
            for k, w in enumerate(widths):
                xt = xp.tile([128, RB * w], mybir.dt.float32, tag="x")
                nc.sync.dma_start(out=xt, in_=xin[:, off:off + RB * w])
                if k in argmax_slots:
                    j = argmax_slots.index(k)
                    for rb in range(RB):
                        sl = xt[:, rb * w:(rb + 1) * w]
                        m8 = small.tile([128, 8], mybir.dt.float32, tag="m8")
                        i8 = small.tile([128, 8], mybir.dt.uint32, tag="i8")
                        nc.vector.max(out=m8, in_=sl)
                        nc.vector.max_index(out=i8, in_max=m8, in_values=sl)
                        col = k * RB + rb
                        nc.vector.tensor_copy(out=pool_t[:, col:col + 1],
                                              in_=m8[:, 0:1])
                        jcol = j * RB + rb
                        nc.vector.tensor_copy(out=idx_t[:, jcol:jcol + 1],
                                              in_=i8[:, 0:1])
                else:
                    nc.vector.tensor_reduce(
                        out=pool_t[:, k * RB:(k + 1) * RB],
                        in_=xt.rearrange("p (r w) -> p r w", r=RB),
                        op=mybir.AluOpType.max,
                        axis=mybir.AxisListType.X,
                    )
                off += RB * w
            nc.sync.dma_start(out=pool_out, in_=pool_t)
            if n_arg:
                nc.sync.dma_start(out=idx_out, in_=idx_t)
    nc.compile()
    return nc


def _get_program(widths, argmax_slots):
    key = (tuple(widths), tuple(argmax_slots))
    if key not in _program_cache:
        _program_cache[key] = _build_program(widths, argmax_slots)
    return _program_cache[key]


def _pack_inputs(xv, lengths, slots, widths):
    """Build per-core [128, C] buffers in SBUF layout (partition p holds
    x-rows {p, 128+p, 256+p, 384+p}), padded with -FLT_MAX past each length."""
    ctotal = 4 * sum(widths)
    bufs = [np.empty((128, ctotal), np.float32) for _ in range(NCORES)]
    for k, (sl, w) in enumerate(zip(slots, widths)):
        off = 4 * sum(widths[:k])
        for c, b in enumerate(sl):
            ln = int(lengths[b])
            block = bufs[c][:, off:off + 4 * w].reshape(128, RB, w)
            # xv[b, rb, p, :] -> block[p, rb, :]
            block[:, :, :ln] = xv[b, :, :, :ln].transpose(1, 0, 2)
            if ln < w:
                block[:, :, ln:] = PAD
    return bufs


def kernel(find_contri, code_output, lengths, contri, fc_w):
    fc = int(np.asarray(find_contri))
    co = np.ascontiguousarray(np.asarray(code_output, dtype=np.float32))
    lengths = np.asarray(lengths).astype(np.int64)
    fc_w = np.asarray(fc_w, dtype=np.float32)

    need_argmax = bool(fc)
    slots, widths, argmax_slots = _assign_slots(lengths, need_argmax)
    nc = _get_program(widths, argmax_slots)

    xv = co.reshape(B, RB, 128, S)  # x[b, rb*128+p, j] = xv[b, rb, p, j]
    bufs = _pack_inputs(xv, lengths, slots, widths)
    in_maps = [{"xin": bufs[c]} for c in range(NCORES)]

    res = run_bass_kernel_spmd(nc, in_maps, core_ids=list(range(NCORES)))

    pool = np.empty((B, H), np.float32)
    idx = np.zeros((B, H), np.int64)
    for k, sl in enumerate(slots):
        for c, b in enumerate(sl):
            pr = res.results[c]["pool_out"]  # [128, 64]
            pool[b] = pr[:, k * RB:(k + 1) * RB].T.reshape(H)
            if k in argmax_slots:
                j = argmax_slots.index(k)
                ir = res.results[c]["idx_out"]  # [128, 4*n_arg]
                idx[b] = ir[:, j * RB:(j + 1) * RB].T.reshape(H).astype(np.int64)

    attn = pool @ fc_w.T  # [B, 11] f32
    if not fc:
        return attn

    contri = np.asarray(contri, dtype=np.float32)
    p10 = pool[:10]
    con = np.maximum(p10[:, None, :] * fc_w[None, :, :], 0.0) + np.float32(1e-4)
    con = con / con.sum(axis=2, keepdims=True, dtype=np.float32) * np.float32(100.0)
    total = con.sum(axis=1, dtype=np.float32)  # [10, H]
    ch2 = np.zeros((B, S), np.float32)
    for i in range(10):
        np.add.at(ch2[i], idx[i], total[i])
    contri_out = np.concatenate([contri, ch2[..., None]], axis=2)
    return attn, contri_out
